# revision 4
# baseline (speedup 1.0000x reference)
"""Trainium2 Bass kernel for nn_LinformerProjectionEntireOutImg.

Math: the reference's softmax is over a constant tensor -> uniform 1/64, so
the whole net collapses to a linear pipeline.  With u = n % 128 (capsule
index within a 128-block), pose = n // 128, l = i*8+j:
  Xs[b,u,l]   = sum_pose cp[b, pose*128+u, i, :] @ wc[pose*128+u, :, j]
  pooled[b,p] = sum_{u,l} Xs[b,u,l] * EfU[u,l,p]
  EfU[u,l,p]  = (1/64) * sum_h4 E_proj[u//4, (u%4)*64+l, h4*64+p]
  v[b,p]      = pooled[b,p] + rel[p]
  out[b,o,i,j]= sum_m v[b, i*8+m] * w_next[o, m, j]

Sharding: core c owns u in [16c, 16c+16) (== heads 4c..4c+4), all poses,
all batches.  Each core emits pooled_c (partial over its u's); the unshard
is a tiny host sum followed by the (trivial) rel/w_next affine tail.

The wall-clock of a warm kernel() call is dominated by host prepack and
shipping bytes to the device over the axon tunnel (~120 MB/s), so the
design goals are: near-zero host math (one fused gather+bf16-cast of
current_pose, ~40 ms), minimal shipped bytes (~34 MB), and all layout
shuffles done on-device (DMA-transpose XBAR + block-diag weights built by
DMA scatter).  Stage 1/2 run on the PE in bf16 with fp32 PSUM accumulate.
"""

import os

import numpy as np

_STATE: dict = {}

B, OUT_N, POSE = 32, 64, 64
NCORES = 8


def _build_nc():
    import concourse.mybir as mybir
    from concourse import bacc
    from concourse.tile import TileContext

    f32 = mybir.dt.float32
    bf16 = mybir.dt.bfloat16
    nc = bacc.Bacc()
    # cp slice, laid out (pose, i, b, q, m) so the XBAR transpose below
    # lands [(q,m), pose*256 + i*32 + b] with per-pose contiguous columns.
    CP = nc.dram_tensor("cp", [64, 8, 32, 16, 8], bf16, kind="ExternalInput")
    # w_current slice, (pose, q, m, j)
    WC = nc.dram_tensor("wc", [64, 16, 8, 8], bf16, kind="ExternalInput")
    # folded E_proj slice, rows (i, q, j) x cols p'
    EF = nc.dram_tensor("ef", [1024, 64], bf16, kind="ExternalInput")
    POOLED = nc.dram_tensor("pooled", [32, 64], f32, kind="ExternalOutput")

    with TileContext(nc) as tc:
        with (
            tc.tile_pool(name="cpool", bufs=1) as cpool,
            tc.tile_pool(name="wpool", bufs=1) as wpool,
            tc.tile_pool(name="spool", bufs=1) as spool,
            tc.tile_pool(name="pp", bufs=1, space="PSUM") as pp,
        ):
            # cp -> SBUF via DMA-transpose XBAR: [128=(q,m), 16384=(pose,i,b)]
            # split in two so the second half overlaps the first chain of
            # stage-1 matmuls.
            cpx = cpool.tile([128, 16384], bf16, tag="cpx")
            cpr = CP.rearrange("pose i b q m -> (pose i b) (q m)")
            nc.sync.dma_start(out=cpx[:, 0:8192], in_=cpr[0:8192, :], transpose=True)
            nc.scalar.dma_start(
                out=cpx[:, 8192:16384], in_=cpr[8192:16384, :], transpose=True
            )

            # EF tile: [128=(q,j), 8=i, 64=p']
            eft = spool.tile([128, 512], bf16, tag="eft")
            eft3 = eft.rearrange("p (i e) -> p i e", i=8)
            nc.scalar.dma_start(
                out=eft3[:], in_=EF.rearrange("(i qj) p -> qj i p", i=8)
            )

            # Block-diagonal stage-1 weights, built on device:
            # wall[(q,m), pose*128 + q*8 + j] = wc[pose, q, m, j], zero elsewhere.
            wall = wpool.tile([128, 8192], bf16, tag="wall")
            nc.vector.memset(wall[:], 0)
            wallv = wall.rearrange("p (P q j) -> p P q j", P=64, q=16)
            for q in range(16):
                nc.sync.dma_start(
                    out=wallv[q * 8 : (q + 1) * 8, :, q, :],
                    in_=WC[:, q].rearrange("P m j -> m P j"),
                )

            # stage 1: Xs[(q,j), (i,b)] = sum_pose Wblk_pose.T @ A_pose
            # two PSUM accumulation chains so per-matmul ordering waits don't
            # serialize the PE, and chain B only needs the second XBAR half.
            psA = pp.tile([128, 256], f32, tag="psA")
            psB = pp.tile([128, 256], f32, tag="psB")
            for P in range(64):
                tgt = psA if P < 32 else psB
                nc.tensor.matmul(
                    tgt[:],
                    wall[:, P * 128 : (P + 1) * 128],
                    cpx[:, P * 256 : (P + 1) * 256],
                    start=(P % 32 == 0),
                    stop=(P % 32 == 31),
                )
            xh = spool.tile([128, 256], f32, tag="xh")
            nc.vector.tensor_copy(xh[:], psA[:])
            xs = spool.tile([128, 256], bf16, tag="xs")
            nc.vector.tensor_add(xs[:], xh[:], psB[:])

            # stage 2: pooled[b, p'] = sum_i Xs[:, i-cols].T @ EF_i
            pps = pp.tile([32, 64], f32, tag="pooled_ps")
            for i in range(8):
                nc.tensor.matmul(
                    pps[:],
                    xs[:, i * 32 : (i + 1) * 32],
                    eft3[:, i, :],
                    start=(i == 0),
                    stop=(i == 7),
                )
            pooled_sb = spool.tile([32, 64], f32, tag="pooled_sb")
            nc.vector.tensor_copy(pooled_sb[:], pps[:])
            nc.sync.dma_start(out=POOLED[:], in_=pooled_sb[:])
    nc.finalize()
    return nc


def _prepack(current_pose, w_current, E_proj):
    import ml_dtypes

    bf16 = ml_dtypes.bfloat16
    cp5 = np.asarray(current_pose).reshape(32, 64, 128, 8, 8)
    wc4 = np.asarray(w_current).reshape(64, 128, 8, 8)
    Efold = np.asarray(E_proj).reshape(32, 256, 4, 64).sum(axis=2) * (1.0 / 64.0)
    E5 = Efold.reshape(32, 4, 8, 8, 64)  # (nh, s2h, i, j, p')

    in_maps = []
    for c in range(NCORES):
        sl = slice(16 * c, 16 * (c + 1))
        cp_c = np.empty((64, 8, 32, 16, 8), bf16)
        cp_c[:] = cp5[:, :, sl].transpose(1, 3, 0, 2, 4)  # fused gather+cast
        wc_c = np.ascontiguousarray(wc4[:, sl], dtype=bf16)
        ef_c = np.ascontiguousarray(
            E5[4 * c : 4 * (c + 1)].transpose(2, 0, 1, 3, 4), dtype=bf16
        ).reshape(1024, 64)
        in_maps.append({"cp": cp_c, "wc": wc_c, "ef": ef_c})
    return in_maps


def _host_tail(pooled, rel_embedd, w_next):
    v = (pooled + np.asarray(rel_embedd, dtype=np.float32).reshape(1, 64)).reshape(
        32, 8, 8
    )
    wn = np.asarray(w_next, dtype=np.float32)
    # out[b,o,i,j] = sum_m v[b,i,m] * wn[o,m,j]
    out = np.matmul(
        v.reshape(256, 8), wn.transpose(1, 0, 2).reshape(8, 512)
    )  # [(b,i), (o,j)]
    out = (
        out.reshape(32, 8, 64, 8).transpose(0, 2, 1, 3).reshape(32, 1, 64, 64)
    )
    return np.ascontiguousarray(out, dtype=np.float32)


def kernel(current_pose, w_current, w_next, E_proj, rel_embedd):
    from concourse import bass_utils

    if "nc" not in _STATE:
        _STATE["nc"] = _build_nc()
    nc = _STATE["nc"]
    in_maps = _prepack(current_pose, w_current, E_proj)
    trace = os.environ.get("KERNEL_TRACE") == "1"
    res = bass_utils.run_bass_kernel_spmd(
        nc, in_maps, core_ids=list(range(NCORES)), trace=trace
    )
    _STATE["last_result"] = res
    pooled = np.zeros((32, 64), np.float32)
    for c in range(NCORES):
        pooled += res.results[c]["pooled"]
    return _host_tail(pooled, rel_embedd, w_next)
